# revision 8
# baseline (speedup 1.0000x reference)
"""BandSplit kernel for Trainium2 (8 NeuronCores, SPMD data-parallel).

Math: the (deterministic) melbank partitions the 1025 STFT bins into 257
contiguous segments (widths 1/4/8/8/1), all mel weights are 1.0, so

    out[b,c,t,k,o] = sum_{f in seg(k)} sum_i x[b,c,t,f,i]*pre_w[i,f,o] + pre_b[k,o]

Sharding: data-parallel over the 8 (b,c) pairs, one per core.
Per core: 256 tokens; out (256, 257, 128) -> memory bound.

Device strategy (v2, bf16 I/O): inputs packed to bf16 on host (~2.5 MB
reads/core), per-band segment matmuls on the PE packed 2-4 bands per
matmul as block-diagonal rhs (K = sum 2w + 1 bias ones-row), bf16 in /
fp32 PSUM accumulate. PSUM -> SBUF copies cast fp32 -> bf16 and span
2 PSUM banks (FD=1024) to amortize the copy-engine fixed cost; copies
are assigned block-wise (all-DVE or all-ACT blocks) so each block's
output DMA (sync ring for DVE blocks, scalar ring for ACT blocks)
never waits cross-engine on its own queue. Output written as bf16
(16.8 MB/core instead of 33.7) and cast back to fp32 on host; total
HBM traffic per core ~19.3 MB vs 38.7 MB for the fp32 kernel.
Rel err ~1e-3 (bf16 rounding), well inside the 2e-2 gate.
"""

import numpy as np
import ml_dtypes

import concourse.bacc as bacc
import concourse.mybir as mybir
from concourse.tile import TileContext
from concourse.bass_utils import run_bass_kernel_spmd

BF16 = np.dtype(ml_dtypes.bfloat16)

# ---------------------------------------------------------------- structure

B, C, T, NF, IN_CH = 4, 2, 256, 1025, 2
N_BANDS, OUT_CH = 257, 128
N_CORES = 8
TOK = 256           # tokens per core (= T; one (b,c) pair per core)
HALVES = 2          # 128-token tiles


def _segments():
    segs = []
    for k in range(N_BANDS):
        if k < 128:
            segs.append((k, 1))
        elif k < 160:
            segs.append((128 + 4 * (k - 128), 4))
        elif k < 192:
            segs.append((256 + 8 * (k - 160), 8))
        elif k < 256:
            segs.append((512 + 8 * (k - 192), 8))
        else:
            segs.append((1024, 1))
    return segs


SEGS = _segments()


def _build_plan():
    """Matmul descriptors: bands (2-4 contiguous), x-group g, 32-aligned
    partition offset, K rows (sum 2w + ones bias row), N out cols, W region
    col start."""
    plan = []
    for a in range(33):  # class A: width-1 bands 0..127 (K=9) + band 256 (K=3)
        bands = [256] if a == 32 else list(range(4 * a, 4 * a + 4))
        plan.append(dict(
            bands=bands, g=a // 4, off=32 * (a % 4),
            K=sum(2 * SEGS[k][1] for k in bands) + 1,
            N=128 * len(bands),
            wcol=512 * (a // 4) if a < 32 else 4096,
        ))
    for b in range(8):   # class B: width-4 bands 128..159 (K=33)
        bands = list(range(128 + 4 * b, 128 + 4 * b + 4))
        plan.append(dict(
            bands=bands, g=9 + b // 2, off=64 * (b % 2),
            K=33, N=512,
            wcol=4224 + 512 * (b // 2),
        ))
    for c in range(48):  # class C: width-8 bands 160..255 (K=33)
        bands = [160 + 2 * c, 160 + 2 * c + 1]
        plan.append(dict(
            bands=bands, g=13 + c // 2, off=64 * (c % 2),
            K=33, N=256,
            wcol=4224 + 2048 + 256 * (c // 2),
        ))
    return plan


PLAN = _build_plan()
NG = 37                      # x column groups
XCOLS = NG * TOK             # 9472
WCOLS = 4224 + 2048 + 6144   # 12416

# Blocks: each block is a list of PLAN indices with the SAME partition
# offset; one staging tile + one output DMA per block. Ordered so early
# blocks depend only on the first-loaded input regions.


def _build_blocks():
    A = lambda off: [a for a in range(32) if PLAN[a]["off"] == off]
    B0 = [i for i in range(33, 41) if PLAN[i]["off"] == 0]
    B64 = [i for i in range(33, 41) if PLAN[i]["off"] == 64]
    C0 = [i for i in range(41, 89) if PLAN[i]["off"] == 0]
    C64 = [i for i in range(41, 89) if PLAN[i]["off"] == 64]
    return [
        A(0) + [32], A(32), A(64), A(96),
        B0, C0[:16], C0[16:],
        B64, C64[:16], C64[16:],
    ]


_BLOCKS = _build_blocks()


def _copy_groups(mms):
    """Split a block's mms into PSUM-tile groups of total N <= 1024
    (one fp32->bf16 copy instruction per group, spanning 2 PSUM banks)."""
    groups, cur, n = [], [], 0
    for i in mms:
        if n + PLAN[i]["N"] > 1024:
            groups.append(cur)
            cur, n = [], 0
        cur.append(i)
        n += PLAN[i]["N"]
    if cur:
        groups.append(cur)
    return groups


# Block pairs: the two blocks of a pair have different partition offsets
# (so consecutive matmuls alternate PE row groups -> LDWEIGHTS of the next
# mm overlaps the running mm) and are drained by different copy engines in
# parallel (block 0 -> DVE -> sync ring, block 1 -> ACT -> scalar ring).
_PAIRS = [(0, 1), (2, 3), (4, 7), (5, 8), (6, 9)]

# (h, block, flat output elem offset, ntot) in emission order; half 1
# swaps copy-engine roles (balancing both engines); both halves end on the
# small C blocks so the kernel drains on 512 KB DMAs.
_OBLOCKS = []
_o = 0
for _h in range(HALVES):
    for _pa, _pb in _PAIRS:
        for _bi in (_pa, _pb):
            _m = _BLOCKS[_bi]
            ntot = sum(PLAN[i]["N"] for i in _m)
            _OBLOCKS.append((_h, _m, _o, ntot))
            _o += 128 * ntot
OELEMS = _o  # == TOK * N_BANDS * OUT_CH

# partition-sliced load regions (row_lo, row_hi, col_lo, col_hi), skipping
# the 32-alignment padding rows. class A x: cols 0..9*256; B/C: rest.
_XLOADS = [
    (0, 9, 0, 2304),
    (32, 41, 0, 2304),
    (64, 73, 0, 2304),
    (96, 105, 0, 2304),
    (0, 33, 2304, XCOLS),
    (64, 97, 2304, XCOLS),
]
_WLOADS = [
    (0, 9, 0, 4224),
    (32, 41, 0, 4096),
    (64, 73, 0, 4096),
    (96, 105, 0, 4096),
    (0, 33, 4224, WCOLS),
    (64, 97, 4224, WCOLS),
]


def _xmm_index():
    """Fancy-index arrays to build x_mm from xt (2050, TOK)."""
    src, dstg, dstr, og, orow = [], [], [], [], []
    for mm in PLAN:
        r = 0
        for k in mm["bands"]:
            f0, w = SEGS[k]
            for l in range(w):
                for i in range(IN_CH):
                    src.append((f0 + l) * 2 + i)
                    dstg.append(mm["g"])
                    dstr.append(mm["off"] + r)
                    r += 1
        og.append(mm["g"])
        orow.append(mm["off"] + r)
    return (np.array(src), np.array(dstg), np.array(dstr),
            np.array(og), np.array(orow))


_XSRC, _XDG, _XDR, _XOG, _XOR = _xmm_index()

# ---------------------------------------------------------------- host prep


def _build_wmm(pre_w, pre_b):
    """(128, WCOLS) bf16: per-mm block-diagonal weights + bias ones-row."""
    wmm = np.zeros((128, WCOLS), dtype=np.float32)
    for mm in PLAN:
        off, wc = mm["off"], mm["wcol"]
        r = 0
        for j, k in enumerate(mm["bands"]):
            f0, w = SEGS[k]
            cols = slice(wc + 128 * j, wc + 128 * (j + 1))
            for l in range(w):
                for i in range(IN_CH):
                    wmm[off + r, cols] = pre_w[i, f0 + l, :]
                    r += 1
            wmm[off + mm["K"] - 1, cols] = pre_b[k, :]
    return wmm.astype(BF16)


def _build_xmm(x_core):
    """x_core (TOK, NF, IN_CH) -> (128, XCOLS) bf16 packed lhsT layout."""
    xt = np.ascontiguousarray(x_core.reshape(TOK, NF * IN_CH).T)  # (2050, TOK)
    xmm = np.zeros((NG, 128, TOK), dtype=np.float32)
    xmm[_XDG, _XDR, :] = xt[_XSRC, :]
    xmm[_XOG, _XOR, :] = 1.0
    return np.ascontiguousarray(
        xmm.transpose(1, 0, 2)).reshape(128, XCOLS).astype(BF16)


def _assemble(out_flat):
    """flat device output (bf16) -> (TOK, N_BANDS, OUT_CH) fp32."""
    oc = np.empty((TOK, N_BANDS, OUT_CH), dtype=np.float32)
    for h, block, o, ntot in _OBLOCKS:
        blk = out_flat[o:o + 128 * ntot].reshape(128, ntot)
        c = 0
        for i in block:
            mm = PLAN[i]
            nb = len(mm["bands"])
            k0 = mm["bands"][0]
            oc[h * 128:(h + 1) * 128, k0:k0 + nb, :] = (
                blk[:, c:c + mm["N"]].reshape(128, nb, OUT_CH))
            c += mm["N"]
    return oc


# ---------------------------------------------------------------- device

_PROGRAM = None


def _build_program():
    global _PROGRAM
    if _PROGRAM is not None:
        return _PROGRAM

    nc = bacc.Bacc("TRN2", target_bir_lowering=False)
    f32 = mybir.dt.float32
    bf16 = mybir.dt.bfloat16
    xin = nc.dram_tensor("xmm", [128, XCOLS], bf16, kind="ExternalInput")
    win = nc.dram_tensor("wmm", [128, WCOLS], bf16, kind="ExternalInput")
    out = nc.dram_tensor("out", [OELEMS], bf16, kind="ExternalOutput")

    with TileContext(nc) as tc:
        with (
            tc.tile_pool(name="xw", bufs=1) as xw_pool,
            tc.tile_pool(name="stage", bufs=6) as stage_pool,
            tc.tile_pool(name="psum", bufs=3, space="PSUM") as psum_pool,
            tc.tile_pool(name="warm", bufs=1, space="PSUM") as warm_pool,
        ):
            # scratch PSUM bank for PE keep-warm filler matmuls (never read):
            # without them the PE duty cycle is ~50% (drain-paced pipeline)
            # and the HAM clock gate keeps the PE at 1.2 GHz all kernel.
            wt = warm_pool.tile([128, 512], f32, tag="warm")
            x_sb = xw_pool.tile([128, XCOLS], bf16, tag="x")
            w_sb = xw_pool.tile([128, WCOLS], bf16, tag="w")
            for r0, r1, c0, c1 in _XLOADS:
                nc.sync.dma_start(out=x_sb[r0:r1, c0:c1],
                                  in_=xin.ap()[r0:r1, c0:c1])
            for r0, r1, c0, c1 in _WLOADS:
                nc.scalar.dma_start(out=w_sb[r0:r1, c0:c1],
                                    in_=win.ap()[r0:r1, c0:c1])

            def emit_mm(ps, pc, i, tcol):
                mm = PLAN[i]
                off, K, N = mm["off"], mm["K"], mm["N"]
                gcol = mm["g"] * TOK + tcol
                nc.tensor.matmul(
                    ps[:, pc:pc + N],
                    x_sb[off:off + K, gcol:gcol + 128],
                    w_sb[off:off + K, mm["wcol"]:mm["wcol"] + N],
                    start=True, stop=True,
                    tile_position=(off, 0),
                )
                return pc + N

            ob = iter(_OBLOCKS)
            for h in range(HALVES):
                for pi in range(len(_PAIRS)):
                    ha, hb = next(ob), next(ob)
                    (_, blkA, oA, ntA) = ha
                    (_, blkB, oB, ntB) = hb
                    # DVE drains block A / ACT drains block B in half 0;
                    # swapped in half 1 so both engines see equal work.
                    a_dve = (h == 0)
                    tcol = h * 128
                    sbA = stage_pool.tile([128, ntA], bf16, tag="st")
                    sbB = stage_pool.tile([128, ntB], bf16, tag="st")
                    gA, gB = _copy_groups(blkA), _copy_groups(blkB)
                    cA = cB = 0
                    for gi in range(max(len(gA), len(gB))):
                        grpA = gA[gi] if gi < len(gA) else []
                        grpB = gB[gi] if gi < len(gB) else []
                        psA = psum_pool.tile([128, 1024], f32, tag="ps",
                                             name="psA") if grpA else None
                        psB = psum_pool.tile([128, 1024], f32, tag="ps",
                                             name="psB") if grpB else None
                        pcA = pcB = 0
                        for mi in range(max(len(grpA), len(grpB))):
                            if mi < len(grpA):
                                pcA = emit_mm(psA, pcA, grpA[mi], tcol)
                            if mi < len(grpB):
                                pcB = emit_mm(psB, pcB, grpB[mi], tcol)
                        # keep-warm filler: runs while the next group's
                        # PSUM tile is still being drained
                        nc.tensor.matmul(
                            wt[:, 0:256],
                            x_sb[0:1, 0:128], w_sb[0:1, 0:256],
                            start=True, stop=True,
                            tile_position=(0, 0),
                        )
                        if grpA:
                            dst = sbA[:, cA:cA + pcA]
                            if a_dve:
                                nc.vector.tensor_copy(dst, psA[:, 0:pcA])
                            else:
                                nc.scalar.copy(dst, psA[:, 0:pcA])
                            cA += pcA
                        if grpB:
                            dst = sbB[:, cB:cB + pcB]
                            if a_dve:
                                nc.scalar.copy(dst, psB[:, 0:pcB])
                            else:
                                nc.vector.tensor_copy(dst, psB[:, 0:pcB])
                            cB += pcB
                    ringA = nc.sync if a_dve else nc.scalar
                    ringB = nc.scalar if a_dve else nc.sync
                    ringA.dma_start(
                        out=out.ap()[oA:oA + 128 * ntA]
                            .rearrange("(p n) -> p n", n=ntA),
                        in_=sbA[:],
                    )
                    ringB.dma_start(
                        out=out.ap()[oB:oB + 128 * ntB]
                            .rearrange("(p n) -> p n", n=ntB),
                        in_=sbB[:],
                    )

    nc.compile()
    _PROGRAM = nc
    return nc


# ---------------------------------------------------------------- entry

LAST_RESULTS = None  # BassKernelResults of the most recent run (for test.py)


def kernel(x, pre_w, pre_b, _trace=False):
    global LAST_RESULTS
    x = np.asarray(x, dtype=np.float32)
    pre_w = np.asarray(pre_w, dtype=np.float32)
    pre_b = np.asarray(pre_b, dtype=np.float32)
    assert x.shape == (B, C, T, NF, IN_CH), x.shape

    nc = _build_program()
    wmm = _build_wmm(pre_w, pre_b)
    in_maps = []
    for core in range(N_CORES):
        b_, c_ = divmod(core, C)
        in_maps.append({"xmm": _build_xmm(x[b_, c_]), "wmm": wmm})

    res = run_bass_kernel_spmd(
        nc, in_maps, core_ids=list(range(N_CORES)), trace=_trace,
    )
    LAST_RESULTS = res

    out = np.empty((B, C, T, N_BANDS, OUT_CH), dtype=np.float32)
    for core in range(N_CORES):
        b_, c_ = divmod(core, C)
        out[b_, c_] = _assemble(res.results[core]["out"])
    return out


# revision 11
# speedup vs baseline: 1.0489x; 1.0489x over previous
"""BandSplit kernel for Trainium2 (8 NeuronCores, SPMD data-parallel).

Math: the (deterministic) melbank partitions the 1025 STFT bins into 257
contiguous segments (widths 1/4/8/8/1), all mel weights are 1.0, so

    out[b,c,t,k,o] = sum_{f in seg(k)} sum_i x[b,c,t,f,i]*pre_w[i,f,o] + pre_b[k,o]

Sharding: data-parallel over the 8 (b,c) pairs, one per core.
Per core: 256 tokens; out (256, 257, 128) -> memory bound.

Device strategy (v2, bf16 I/O): inputs packed to bf16 on host (~2.5 MB
reads/core), per-band segment matmuls on the PE packed 2-4 bands per
matmul as block-diagonal rhs (K = sum 2w + 1 bias ones-row), bf16 in /
fp32 PSUM accumulate. PSUM -> SBUF copies cast fp32 -> bf16 and span
2 PSUM banks (FD=1024) to amortize the copy-engine fixed cost; copies
are assigned block-wise (all-DVE or all-ACT blocks) so each block's
output DMA (sync ring for DVE blocks, scalar ring for ACT blocks)
never waits cross-engine on its own queue. Output written as bf16
(16.8 MB/core instead of 33.7) and cast back to fp32 on host; total
HBM traffic per core ~19.3 MB vs 38.7 MB for the fp32 kernel.
Rel err ~1e-3 (bf16 rounding), well inside the 2e-2 gate.
"""

import numpy as np
import ml_dtypes

import concourse.bacc as bacc
import concourse.mybir as mybir
from concourse.tile import TileContext
from concourse.bass_utils import run_bass_kernel_spmd

BF16 = np.dtype(ml_dtypes.bfloat16)

# ---------------------------------------------------------------- structure

B, C, T, NF, IN_CH = 4, 2, 256, 1025, 2
N_BANDS, OUT_CH = 257, 128
N_CORES = 8
TOK = 256           # tokens per core (= T; one (b,c) pair per core)
HALVES = 2          # 128-token tiles


def _segments():
    segs = []
    for k in range(N_BANDS):
        if k < 128:
            segs.append((k, 1))
        elif k < 160:
            segs.append((128 + 4 * (k - 128), 4))
        elif k < 192:
            segs.append((256 + 8 * (k - 160), 8))
        elif k < 256:
            segs.append((512 + 8 * (k - 192), 8))
        else:
            segs.append((1024, 1))
    return segs


SEGS = _segments()


def _build_plan():
    """Matmul descriptors: bands (2-4 contiguous), x-group g, 32-aligned
    partition offset, K rows (sum 2w + ones bias row), N out cols, W region
    col start."""
    plan = []
    for a in range(33):  # class A: width-1 bands 0..127 (K=9) + band 256 (K=3)
        bands = [256] if a == 32 else list(range(4 * a, 4 * a + 4))
        plan.append(dict(
            bands=bands, g=a // 4, off=32 * (a % 4),
            K=sum(2 * SEGS[k][1] for k in bands) + 1,
            N=128 * len(bands),
            wcol=512 * (a // 4) if a < 32 else 4096,
        ))
    for b in range(8):   # class B: width-4 bands 128..159 (K=33)
        bands = list(range(128 + 4 * b, 128 + 4 * b + 4))
        plan.append(dict(
            bands=bands, g=9 + b // 2, off=64 * (b % 2),
            K=33, N=512,
            wcol=4224 + 512 * (b // 2),
        ))
    for c in range(48):  # class C: width-8 bands 160..255 (K=33)
        bands = [160 + 2 * c, 160 + 2 * c + 1]
        plan.append(dict(
            bands=bands, g=13 + c // 2, off=64 * (c % 2),
            K=33, N=256,
            wcol=4224 + 2048 + 256 * (c // 2),
        ))
    return plan


PLAN = _build_plan()
NG = 37                      # x column groups
XCOLS = NG * TOK             # 9472
WCOLS = 4224 + 2048 + 6144   # 12416

# Blocks: each block is a list of PLAN indices with the SAME partition
# offset; one staging tile + one output DMA per block. Ordered so early
# blocks depend only on the first-loaded input regions.


def _build_blocks():
    A = lambda off: [a for a in range(32) if PLAN[a]["off"] == off]
    B0 = [i for i in range(33, 41) if PLAN[i]["off"] == 0]
    B64 = [i for i in range(33, 41) if PLAN[i]["off"] == 64]
    C0 = [i for i in range(41, 89) if PLAN[i]["off"] == 0]
    C64 = [i for i in range(41, 89) if PLAN[i]["off"] == 64]
    return [
        A(0) + [32], A(32), A(64), A(96),
        B0, C0[:16], C0[16:],
        B64, C64[:16], C64[16:],
    ]


_BLOCKS = _build_blocks()


def _copy_groups(mms):
    """Split a block's mms into PSUM-tile groups of total N <= 1024
    (one fp32->bf16 copy instruction per group, spanning 2 PSUM banks)."""
    groups, cur, n = [], [], 0
    for i in mms:
        if n + PLAN[i]["N"] > 1024:
            groups.append(cur)
            cur, n = [], 0
        cur.append(i)
        n += PLAN[i]["N"]
    if cur:
        groups.append(cur)
    return groups


# Block pairs: the two blocks of a pair have different partition offsets
# (so consecutive matmuls alternate PE row groups -> LDWEIGHTS of the next
# mm overlaps the running mm) and are drained by different copy engines in
# parallel (block 0 -> DVE -> sync ring, block 1 -> ACT -> scalar ring).
_PAIRS = [(0, 1), (2, 3), (4, 7), (5, 8), (6, 9)]

# (h, block, flat output elem offset, ntot) in emission order; half 1
# swaps copy-engine roles (balancing both engines); both halves end on the
# small C blocks so the kernel drains on 512 KB DMAs.
_OBLOCKS = []
_o = 0
for _h in range(HALVES):
    for _pa, _pb in _PAIRS:
        for _bi in (_pa, _pb):
            _m = _BLOCKS[_bi]
            ntot = sum(PLAN[i]["N"] for i in _m)
            _OBLOCKS.append((_h, _m, _o, ntot))
            _o += 128 * ntot
OELEMS = _o  # == TOK * N_BANDS * OUT_CH

# partition-sliced load regions (row_lo, row_hi, col_lo, col_hi), skipping
# the 32-alignment padding rows. class A x: cols 0..9*256; B/C: rest.
_XLOADS = [
    (0, 9, 0, 2304),
    (32, 41, 0, 2304),
    (64, 73, 0, 2304),
    (96, 105, 0, 2304),
    (0, 33, 2304, XCOLS),
    (64, 97, 2304, XCOLS),
]
_WLOADS = [
    (0, 9, 0, 4224),
    (32, 41, 0, 4096),
    (64, 73, 0, 4096),
    (96, 105, 0, 4096),
    (0, 33, 4224, WCOLS),
    (64, 97, 4224, WCOLS),
]


def _xmm_index():
    """Fancy-index arrays to build x_mm from xt (2050, TOK)."""
    src, dstg, dstr, og, orow = [], [], [], [], []
    for mm in PLAN:
        r = 0
        for k in mm["bands"]:
            f0, w = SEGS[k]
            for l in range(w):
                for i in range(IN_CH):
                    src.append((f0 + l) * 2 + i)
                    dstg.append(mm["g"])
                    dstr.append(mm["off"] + r)
                    r += 1
        og.append(mm["g"])
        orow.append(mm["off"] + r)
    return (np.array(src), np.array(dstg), np.array(dstr),
            np.array(og), np.array(orow))


_XSRC, _XDG, _XDR, _XOG, _XOR = _xmm_index()

# ---------------------------------------------------------------- host prep


def _build_wmm(pre_w, pre_b):
    """(128, WCOLS) bf16: per-mm block-diagonal weights + bias ones-row."""
    wmm = np.zeros((128, WCOLS), dtype=np.float32)
    for mm in PLAN:
        off, wc = mm["off"], mm["wcol"]
        r = 0
        for j, k in enumerate(mm["bands"]):
            f0, w = SEGS[k]
            cols = slice(wc + 128 * j, wc + 128 * (j + 1))
            for l in range(w):
                for i in range(IN_CH):
                    wmm[off + r, cols] = pre_w[i, f0 + l, :]
                    r += 1
            wmm[off + mm["K"] - 1, cols] = pre_b[k, :]
    return wmm.astype(BF16)


def _build_xmm(x_core):
    """x_core (TOK, NF, IN_CH) -> (128, XCOLS) bf16 packed lhsT layout."""
    xt = np.ascontiguousarray(x_core.reshape(TOK, NF * IN_CH).T)  # (2050, TOK)
    xmm = np.zeros((NG, 128, TOK), dtype=np.float32)
    xmm[_XDG, _XDR, :] = xt[_XSRC, :]
    xmm[_XOG, _XOR, :] = 1.0
    return np.ascontiguousarray(
        xmm.transpose(1, 0, 2)).reshape(128, XCOLS).astype(BF16)


def _assemble(out_flat):
    """flat device output (bf16) -> (TOK, N_BANDS, OUT_CH) fp32."""
    oc = np.empty((TOK, N_BANDS, OUT_CH), dtype=np.float32)
    for h, block, o, ntot in _OBLOCKS:
        blk = out_flat[o:o + 128 * ntot].reshape(128, ntot)
        c = 0
        for i in block:
            mm = PLAN[i]
            nb = len(mm["bands"])
            k0 = mm["bands"][0]
            oc[h * 128:(h + 1) * 128, k0:k0 + nb, :] = (
                blk[:, c:c + mm["N"]].reshape(128, nb, OUT_CH))
            c += mm["N"]
    return oc


# ---------------------------------------------------------------- device

_PROGRAM = None


def _build_program():
    global _PROGRAM
    if _PROGRAM is not None:
        return _PROGRAM

    nc = bacc.Bacc("TRN2", target_bir_lowering=False)
    f32 = mybir.dt.float32
    bf16 = mybir.dt.bfloat16
    xin = nc.dram_tensor("xmm", [128, XCOLS], bf16, kind="ExternalInput")
    win = nc.dram_tensor("wmm", [128, WCOLS], bf16, kind="ExternalInput")
    out = nc.dram_tensor("out", [OELEMS], bf16, kind="ExternalOutput")

    with TileContext(nc) as tc:
        with (
            tc.tile_pool(name="xw", bufs=1) as xw_pool,
            tc.tile_pool(name="stage", bufs=6) as stage_pool,
            tc.tile_pool(name="psum", bufs=3, space="PSUM") as psum_pool,
            tc.tile_pool(name="warm", bufs=1, space="PSUM") as warm_pool,
        ):
            # PE warm-up burst: 9 back-to-back dummy matmuls reading a
            # memset scratch tile (no load dependency), running during the
            # input-load window into a dedicated scratch PSUM bank that is
            # never read. A >=3.4us continuous PE-busy stretch flips the
            # HAM clock gate 1.2 -> 2.4 GHz before the first real matmul;
            # steady-state drain-paced idles (~1us) are too short to
            # re-throttle.
            zsrc = xw_pool.tile([128, 640], mybir.dt.bfloat16, tag="z")
            nc.vector.memset(zsrc[:], 0.0)
            wt = warm_pool.tile([128, 512], f32, tag="warm")
            for wi in range(9):
                nc.tensor.matmul(
                    wt[:],
                    zsrc[0:1, 0:128],
                    zsrc[0:1, 128:640],
                    start=True, stop=True,
                    tile_position=(0, 0),
                )
            x_sb = xw_pool.tile([128, XCOLS], bf16, tag="x")
            w_sb = xw_pool.tile([128, WCOLS], bf16, tag="w")
            for r0, r1, c0, c1 in _XLOADS:
                nc.sync.dma_start(out=x_sb[r0:r1, c0:c1],
                                  in_=xin.ap()[r0:r1, c0:c1])
            for r0, r1, c0, c1 in _WLOADS:
                nc.scalar.dma_start(out=w_sb[r0:r1, c0:c1],
                                    in_=win.ap()[r0:r1, c0:c1])

            def emit_mm(ps, pc, i, tcol):
                mm = PLAN[i]
                off, K, N = mm["off"], mm["K"], mm["N"]
                gcol = mm["g"] * TOK + tcol
                nc.tensor.matmul(
                    ps[:, pc:pc + N],
                    x_sb[off:off + K, gcol:gcol + 128],
                    w_sb[off:off + K, mm["wcol"]:mm["wcol"] + N],
                    start=True, stop=True,
                    tile_position=(off, 0),
                )
                return pc + N

            ob = iter(_OBLOCKS)
            for h in range(HALVES):
                for pi in range(len(_PAIRS)):
                    ha, hb = next(ob), next(ob)
                    (_, blkA, oA, ntA) = ha
                    (_, blkB, oB, ntB) = hb
                    # DVE drains block A / ACT drains block B in half 0;
                    # swapped in half 1 so both engines see equal work.
                    a_dve = (h == 0)
                    tcol = h * 128
                    sbA = stage_pool.tile([128, ntA], bf16, tag="st")
                    sbB = stage_pool.tile([128, ntB], bf16, tag="st")
                    gA, gB = _copy_groups(blkA), _copy_groups(blkB)
                    cA = cB = 0
                    for gi in range(max(len(gA), len(gB))):
                        grpA = gA[gi] if gi < len(gA) else []
                        grpB = gB[gi] if gi < len(gB) else []
                        psA = psum_pool.tile([128, 1024], f32, tag="ps",
                                             name="psA") if grpA else None
                        psB = psum_pool.tile([128, 1024], f32, tag="ps",
                                             name="psB") if grpB else None
                        pcA = pcB = 0
                        for mi in range(max(len(grpA), len(grpB))):
                            if mi < len(grpA):
                                pcA = emit_mm(psA, pcA, grpA[mi], tcol)
                            if mi < len(grpB):
                                pcB = emit_mm(psB, pcB, grpB[mi], tcol)
                        if grpA:
                            dst = sbA[:, cA:cA + pcA]
                            if a_dve:
                                nc.vector.tensor_copy(dst, psA[:, 0:pcA])
                            else:
                                nc.scalar.copy(dst, psA[:, 0:pcA])
                            cA += pcA
                        if grpB:
                            dst = sbB[:, cB:cB + pcB]
                            if a_dve:
                                nc.scalar.copy(dst, psB[:, 0:pcB])
                            else:
                                nc.vector.tensor_copy(dst, psB[:, 0:pcB])
                            cB += pcB
                    ringA = nc.sync if a_dve else nc.scalar
                    ringB = nc.scalar if a_dve else nc.sync
                    ringA.dma_start(
                        out=out.ap()[oA:oA + 128 * ntA]
                            .rearrange("(p n) -> p n", n=ntA),
                        in_=sbA[:],
                    )
                    ringB.dma_start(
                        out=out.ap()[oB:oB + 128 * ntB]
                            .rearrange("(p n) -> p n", n=ntB),
                        in_=sbB[:],
                    )

    nc.compile()
    _PROGRAM = nc
    return nc


# ---------------------------------------------------------------- entry

LAST_RESULTS = None  # BassKernelResults of the most recent run (for test.py)


def kernel(x, pre_w, pre_b, _trace=False):
    global LAST_RESULTS
    x = np.asarray(x, dtype=np.float32)
    pre_w = np.asarray(pre_w, dtype=np.float32)
    pre_b = np.asarray(pre_b, dtype=np.float32)
    assert x.shape == (B, C, T, NF, IN_CH), x.shape

    nc = _build_program()
    wmm = _build_wmm(pre_w, pre_b)
    in_maps = []
    for core in range(N_CORES):
        b_, c_ = divmod(core, C)
        in_maps.append({"xmm": _build_xmm(x[b_, c_]), "wmm": wmm})

    res = run_bass_kernel_spmd(
        nc, in_maps, core_ids=list(range(N_CORES)), trace=_trace,
    )
    LAST_RESULTS = res

    out = np.empty((B, C, T, N_BANDS, OUT_CH), dtype=np.float32)
    for core in range(N_CORES):
        b_, c_ = divmod(core, C)
        out[b_, c_] = _assemble(res.results[core]["out"])
    return out


# revision 13
# speedup vs baseline: 1.1489x; 1.0954x over previous
"""BandSplit kernel for Trainium2 (8 NeuronCores, SPMD data-parallel).

Math: the (deterministic) melbank partitions the 1025 STFT bins into 257
contiguous segments (widths 1/4/8/8/1), all mel weights are 1.0, so

    out[b,c,t,k,o] = sum_{f in seg(k)} sum_i x[b,c,t,f,i]*pre_w[i,f,o] + pre_b[k,o]

Sharding: data-parallel over the 8 (b,c) pairs, one per core.
Per core: 256 tokens; out (256, 257, 128) -> memory bound.

Device strategy (v2, bf16 I/O): inputs packed to bf16 on host (~2.5 MB
reads/core), per-band segment matmuls on the PE packed 2-4 bands per
matmul as block-diagonal rhs (K = sum 2w + 1 bias ones-row), bf16 in /
fp32 PSUM accumulate. PSUM -> SBUF copies cast fp32 -> bf16 and span
2 PSUM banks (FD=1024) to amortize the copy-engine fixed cost; copies
are assigned block-wise (all-DVE or all-ACT blocks) so each block's
output DMA (sync ring for DVE blocks, scalar ring for ACT blocks)
never waits cross-engine on its own queue. Output written as bf16
(16.8 MB/core instead of 33.7) and cast back to fp32 on host; total
HBM traffic per core ~19.3 MB vs 38.7 MB for the fp32 kernel.
Rel err ~1e-3 (bf16 rounding), well inside the 2e-2 gate.
"""

import numpy as np
import ml_dtypes

import concourse.bacc as bacc
import concourse.mybir as mybir
from concourse.tile import TileContext
from concourse.bass_utils import run_bass_kernel_spmd

BF16 = np.dtype(ml_dtypes.bfloat16)

# ---------------------------------------------------------------- structure

B, C, T, NF, IN_CH = 4, 2, 256, 1025, 2
N_BANDS, OUT_CH = 257, 128
N_CORES = 8
TOK = 256           # tokens per core (= T; one (b,c) pair per core)
HALVES = 2          # 128-token tiles


def _segments():
    segs = []
    for k in range(N_BANDS):
        if k < 128:
            segs.append((k, 1))
        elif k < 160:
            segs.append((128 + 4 * (k - 128), 4))
        elif k < 192:
            segs.append((256 + 8 * (k - 160), 8))
        elif k < 256:
            segs.append((512 + 8 * (k - 192), 8))
        else:
            segs.append((1024, 1))
    return segs


SEGS = _segments()


def _build_plan():
    """Matmul descriptors: bands (2-4 contiguous), x-group g, 32-aligned
    partition offset, K rows (sum 2w + ones bias row), N out cols, W region
    col start."""
    plan = []
    for a in range(33):  # class A: width-1 bands 0..127 (K=9) + band 256 (K=3)
        bands = [256] if a == 32 else list(range(4 * a, 4 * a + 4))
        plan.append(dict(
            bands=bands, g=a // 4, off=32 * (a % 4),
            K=sum(2 * SEGS[k][1] for k in bands) + 1,
            N=128 * len(bands),
            wcol=512 * (a // 4) if a < 32 else 4096,
        ))
    for b in range(8):   # class B: width-4 bands 128..159 (K=33)
        bands = list(range(128 + 4 * b, 128 + 4 * b + 4))
        plan.append(dict(
            bands=bands, g=9 + b // 2, off=64 * (b % 2),
            K=33, N=512,
            wcol=4224 + 512 * (b // 2),
        ))
    for c in range(48):  # class C: width-8 bands 160..255 (K=33)
        bands = [160 + 2 * c, 160 + 2 * c + 1]
        plan.append(dict(
            bands=bands, g=13 + c // 2, off=64 * (c % 2),
            K=33, N=256,
            wcol=4224 + 2048 + 256 * (c // 2),
        ))
    return plan


PLAN = _build_plan()
NG = 37                      # x column groups
XCOLS = NG * TOK             # 9472
WCOLS = 4224 + 2048 + 6144   # 12416

# Blocks: each block is a list of PLAN indices with the SAME partition
# offset; one staging tile + one output DMA per block. Ordered so early
# blocks depend only on the first-loaded input regions.


def _build_blocks():
    A = lambda off: [a for a in range(32) if PLAN[a]["off"] == off]
    B0 = [i for i in range(33, 41) if PLAN[i]["off"] == 0]
    B64 = [i for i in range(33, 41) if PLAN[i]["off"] == 64]
    C0 = [i for i in range(41, 89) if PLAN[i]["off"] == 0]
    C64 = [i for i in range(41, 89) if PLAN[i]["off"] == 64]
    return [
        A(0) + [32], A(32), A(64), A(96),
        B0, C0[:16], C0[16:],
        B64, C64[:16], C64[16:],
    ]


_BLOCKS = _build_blocks()


def _copy_groups(mms):
    """Split a block's mms into PSUM-tile groups of total N <= 1024
    (one fp32->bf16 copy instruction per group, spanning 2 PSUM banks)."""
    groups, cur, n = [], [], 0
    for i in mms:
        if n + PLAN[i]["N"] > 1024:
            groups.append(cur)
            cur, n = [], 0
        cur.append(i)
        n += PLAN[i]["N"]
    if cur:
        groups.append(cur)
    return groups


# Block pairs: the two blocks of a pair have different partition offsets
# (so consecutive matmuls alternate PE row groups -> LDWEIGHTS of the next
# mm overlaps the running mm) and are drained by different copy engines in
# parallel (block 0 -> DVE -> sync ring, block 1 -> ACT -> scalar ring).
_PAIRS = [(0, 1), (2, 3), (4, 7), (5, 8), (6, 9)]

# (h, block, flat output elem offset, ntot) in emission order; half 1
# swaps copy-engine roles (balancing both engines); both halves end on the
# small C blocks so the kernel drains on 512 KB DMAs.
_OBLOCKS = []
_o = 0
for _h in range(HALVES):
    for _pa, _pb in _PAIRS:
        for _bi in (_pa, _pb):
            _m = _BLOCKS[_bi]
            ntot = sum(PLAN[i]["N"] for i in _m)
            _OBLOCKS.append((_h, _m, _o, ntot))
            _o += 128 * ntot
OELEMS = _o  # == TOK * N_BANDS * OUT_CH

# partition-sliced load regions (row_lo, row_hi, col_lo, col_hi), skipping
# the 32-alignment padding rows. class A x: cols 0..9*256; B/C: rest.
_XLOADS = [
    (0, 9, 0, 2304),
    (32, 41, 0, 2304),
    (64, 73, 0, 2304),
    (96, 105, 0, 2304),
    (0, 33, 2304, XCOLS),
    (64, 97, 2304, XCOLS),
]
_WLOADS = [
    (0, 9, 0, 4224),
    (32, 41, 0, 4096),
    (64, 73, 0, 4096),
    (96, 105, 0, 4096),
    (0, 33, 4224, WCOLS),
    (64, 97, 4224, WCOLS),
]


def _xmm_index():
    """Fancy-index arrays to build x_mm from xt (2050, TOK)."""
    src, dstg, dstr, og, orow = [], [], [], [], []
    for mm in PLAN:
        r = 0
        for k in mm["bands"]:
            f0, w = SEGS[k]
            for l in range(w):
                for i in range(IN_CH):
                    src.append((f0 + l) * 2 + i)
                    dstg.append(mm["g"])
                    dstr.append(mm["off"] + r)
                    r += 1
        og.append(mm["g"])
        orow.append(mm["off"] + r)
    return (np.array(src), np.array(dstg), np.array(dstr),
            np.array(og), np.array(orow))


_XSRC, _XDG, _XDR, _XOG, _XOR = _xmm_index()

# ---------------------------------------------------------------- host prep


def _build_wmm(pre_w, pre_b):
    """(128, WCOLS) bf16: per-mm block-diagonal weights + bias ones-row."""
    wmm = np.zeros((128, WCOLS), dtype=np.float32)
    for mm in PLAN:
        off, wc = mm["off"], mm["wcol"]
        r = 0
        for j, k in enumerate(mm["bands"]):
            f0, w = SEGS[k]
            cols = slice(wc + 128 * j, wc + 128 * (j + 1))
            for l in range(w):
                for i in range(IN_CH):
                    wmm[off + r, cols] = pre_w[i, f0 + l, :]
                    r += 1
            wmm[off + mm["K"] - 1, cols] = pre_b[k, :]
    return wmm.astype(BF16)


def _build_xmm(x_core):
    """x_core (TOK, NF, IN_CH) -> (128, XCOLS) bf16 packed lhsT layout."""
    xt = np.ascontiguousarray(x_core.reshape(TOK, NF * IN_CH).T)  # (2050, TOK)
    xmm = np.zeros((NG, 128, TOK), dtype=np.float32)
    xmm[_XDG, _XDR, :] = xt[_XSRC, :]
    xmm[_XOG, _XOR, :] = 1.0
    return np.ascontiguousarray(
        xmm.transpose(1, 0, 2)).reshape(128, XCOLS).astype(BF16)


def _assemble(out_flat):
    """flat device output (bf16) -> (TOK, N_BANDS, OUT_CH) fp32."""
    oc = np.empty((TOK, N_BANDS, OUT_CH), dtype=np.float32)
    for h, block, o, ntot in _OBLOCKS:
        blk = out_flat[o:o + 128 * ntot].reshape(128, ntot)
        c = 0
        for i in block:
            mm = PLAN[i]
            nb = len(mm["bands"])
            k0 = mm["bands"][0]
            oc[h * 128:(h + 1) * 128, k0:k0 + nb, :] = (
                blk[:, c:c + mm["N"]].reshape(128, nb, OUT_CH))
            c += mm["N"]
    return oc


# ---------------------------------------------------------------- device

_PROGRAM = None


def _build_program():
    global _PROGRAM
    if _PROGRAM is not None:
        return _PROGRAM

    nc = bacc.Bacc("TRN2", target_bir_lowering=False)
    f32 = mybir.dt.float32
    bf16 = mybir.dt.bfloat16
    xin = nc.dram_tensor("xmm", [128, XCOLS], bf16, kind="ExternalInput")
    win = nc.dram_tensor("wmm", [128, WCOLS], bf16, kind="ExternalInput")
    out = nc.dram_tensor("out", [OELEMS], bf16, kind="ExternalOutput")

    with TileContext(nc) as tc:
        with (
            tc.tile_pool(name="xw", bufs=1) as xw_pool,
            tc.tile_pool(name="stage", bufs=6) as stage_pool,
            tc.tile_pool(name="psum", bufs=4, space="PSUM") as psum_pool,
        ):
            x_sb = xw_pool.tile([128, XCOLS], bf16, tag="x")
            w_sb = xw_pool.tile([128, WCOLS], bf16, tag="w")
            for r0, r1, c0, c1 in _XLOADS:
                nc.sync.dma_start(out=x_sb[r0:r1, c0:c1],
                                  in_=xin.ap()[r0:r1, c0:c1])
            for r0, r1, c0, c1 in _WLOADS:
                nc.scalar.dma_start(out=w_sb[r0:r1, c0:c1],
                                    in_=win.ap()[r0:r1, c0:c1])

            def emit_mm(ps, pc, i, tcol):
                mm = PLAN[i]
                off, K, N = mm["off"], mm["K"], mm["N"]
                gcol = mm["g"] * TOK + tcol
                nc.tensor.matmul(
                    ps[:, pc:pc + N],
                    x_sb[off:off + K, gcol:gcol + 128],
                    w_sb[off:off + K, mm["wcol"]:mm["wcol"] + N],
                    start=True, stop=True,
                    tile_position=(off, 0),
                )
                return pc + N

            ob = iter(_OBLOCKS)
            for h in range(HALVES):
                for pi in range(len(_PAIRS)):
                    ha, hb = next(ob), next(ob)
                    (_, blkA, oA, ntA) = ha
                    (_, blkB, oB, ntB) = hb
                    # DVE drains block A / ACT drains block B in half 0;
                    # swapped in half 1 so both engines see equal work.
                    a_dve = (h == 0)
                    tcol = h * 128
                    sbA = stage_pool.tile([128, ntA], bf16, tag="st")
                    sbB = stage_pool.tile([128, ntB], bf16, tag="st")
                    gA, gB = _copy_groups(blkA), _copy_groups(blkB)
                    cA = cB = 0
                    for gi in range(max(len(gA), len(gB))):
                        grpA = gA[gi] if gi < len(gA) else []
                        grpB = gB[gi] if gi < len(gB) else []
                        psA = psum_pool.tile([128, 1024], f32, tag="ps",
                                             name="psA") if grpA else None
                        psB = psum_pool.tile([128, 1024], f32, tag="ps",
                                             name="psB") if grpB else None
                        pcA = pcB = 0
                        for mi in range(max(len(grpA), len(grpB))):
                            if mi < len(grpA):
                                pcA = emit_mm(psA, pcA, grpA[mi], tcol)
                            if mi < len(grpB):
                                pcB = emit_mm(psB, pcB, grpB[mi], tcol)
                        if grpA:
                            dst = sbA[:, cA:cA + pcA]
                            if a_dve:
                                nc.vector.tensor_copy(dst, psA[:, 0:pcA])
                            else:
                                nc.scalar.copy(dst, psA[:, 0:pcA])
                            cA += pcA
                        if grpB:
                            dst = sbB[:, cB:cB + pcB]
                            if a_dve:
                                nc.scalar.copy(dst, psB[:, 0:pcB])
                            else:
                                nc.vector.tensor_copy(dst, psB[:, 0:pcB])
                            cB += pcB
                    # output DMAs never issue from the copy engines: sync
                    # (HWDGE ring 0) and gpsimd (SWDGE queues) — so the
                    # DVE/ACT copy streams have no DMA-issue bubbles.
                    ringA = nc.sync if a_dve else nc.gpsimd
                    ringB = nc.gpsimd if a_dve else nc.sync
                    ringA.dma_start(
                        out=out.ap()[oA:oA + 128 * ntA]
                            .rearrange("(p n) -> p n", n=ntA),
                        in_=sbA[:],
                    )
                    ringB.dma_start(
                        out=out.ap()[oB:oB + 128 * ntB]
                            .rearrange("(p n) -> p n", n=ntB),
                        in_=sbB[:],
                    )

    nc.compile()
    _PROGRAM = nc
    return nc


# ---------------------------------------------------------------- entry

LAST_RESULTS = None  # BassKernelResults of the most recent run (for test.py)


def kernel(x, pre_w, pre_b, _trace=False):
    global LAST_RESULTS
    x = np.asarray(x, dtype=np.float32)
    pre_w = np.asarray(pre_w, dtype=np.float32)
    pre_b = np.asarray(pre_b, dtype=np.float32)
    assert x.shape == (B, C, T, NF, IN_CH), x.shape

    nc = _build_program()
    wmm = _build_wmm(pre_w, pre_b)
    in_maps = []
    for core in range(N_CORES):
        b_, c_ = divmod(core, C)
        in_maps.append({"xmm": _build_xmm(x[b_, c_]), "wmm": wmm})

    res = run_bass_kernel_spmd(
        nc, in_maps, core_ids=list(range(N_CORES)), trace=_trace,
    )
    LAST_RESULTS = res

    out = np.empty((B, C, T, N_BANDS, OUT_CH), dtype=np.float32)
    for core in range(N_CORES):
        b_, c_ = divmod(core, C)
        out[b_, c_] = _assemble(res.results[core]["out"])
    return out
